# revision 31
# baseline (speedup 1.0000x reference)
"""Trainium2 Bass kernel for the AttLayer pooling module.

Reference computation (per batch b):
    uit  = tanh(x @ W + bias)            # [T, A]
    ait  = exp(uit @ u) * mask           # [T]
    out  = x^T @ (ait / (sum(ait)+EPS))  # [D]

Distribution: pure data parallel, batch dim B=64 sharded across 8 NeuronCores
(8 batches per core). W/b/u are replicated.

Device dataflow per batch (all matmuls bf16 with f32 PSUM accumulation):
  mm1: uitT[a,t] = sum_d W[d,a] * x[t,d]   lhsT = W d-chunk [128,128] (stationary),
                                           rhs = xT d-chunk [128, t-subtile 512]
  tanh(+bias) on ScalarE, PSUM -> SBUF bf16 (bias b is per-partition since a is
  the partition dim of uitT)
  mm2: s[t] (t on partitions!) via lhsT = uitT t-chunk [a=128, 128], rhs = u [a,1]
  exp on ScalarE -> mask multiply + bf16 cast on VectorE -> ait [128, 16]
  denom: ones^T @ ait -> [1,16] -> reduce -> +EPS -> reciprocal
  mm3: out_raw[1, D] += ait[:, j]^T @ xn[t-chunk j]   (accumulate 16 chunks)
  scale by 1/denom on ScalarE -> out row

The contraction dims differ between mm1 (over D) and mm3 (over T), so the two
matmuls need x in transposed resp. natural layout. Both layouts are staged in
DRAM as bf16, which keeps total HBM read bytes identical to a single f32 copy
of x (the memory roofline is unchanged: 32 MiB/core).
"""

import sys

if "/opt/trn_rl_repo" not in sys.path:
    sys.path.insert(0, "/opt/trn_rl_repo")

import numpy as np
import ml_dtypes

import concourse.bass as bass  # noqa: F401  (registers AP machinery)
import concourse.tile as tile
from concourse import bacc, mybir
from concourse.bass import ts
from concourse.bass_utils import run_bass_kernel_spmd

BF16 = mybir.dt.bfloat16
F32 = mybir.dt.float32
AFT = mybir.ActivationFunctionType

EPS = 1e-7

B, T, D, A = 64, 2048, 512, 128
NCORES = 8
BS = B // NCORES          # 8 batches per core
DCH = D // 128            # 4 d-chunks of 128
TJ = T // 128             # 16 t-chunks of 128
TSUB = 512                # t-subtile width for mm1
TS = T // TSUB            # 4 t-subtiles

_NC_CACHE = {}


def _r4k(ap):
    """Reshape a [128, A, B] AP to 4-KiB (2048 bf16) contiguous runs."""
    return ap.rearrange("p a b -> p (a b)").rearrange("p (k r) -> p k r", r=2048)


def _build_nc(repeat=1, mode="full"):
    """mode: 'full' | 'dma' (loads only) | 'compute' (load once, compute loop)."""
    nc = bacc.Bacc("TRN2", target_bir_lowering=False, debug=False)

    # Pre-swizzled host layouts: a load is one linear DRAM scan, contiguous on
    # both the DRAM and SBUF side (maximal DMA descriptors, sequential DRAM).
    #   xt[b, p, c, t] = x[b, t, 128c+p]   (transposed view, d = 128c+p)
    #   xn[b, p, j, d] = x[b, 128j+p, d]   (natural view,    t = 128j+p)
    xt_d = nc.declare_dram_parameter("xt", [BS, 128, DCH, T], BF16, isOutput=False)
    xn_d = nc.declare_dram_parameter("xn", [BS, 128, TJ, D], BF16, isOutput=False)
    mk_d = nc.declare_dram_parameter("maskr", [BS, 128, TJ], F32, isOutput=False)
    w_d = nc.declare_dram_parameter("w", [D, A], BF16, isOutput=False)
    b_d = nc.declare_dram_parameter("b", [A, 1], F32, isOutput=False)
    u_d = nc.declare_dram_parameter("u", [A, 1], BF16, isOutput=False)
    out_d = nc.declare_dram_parameter("out", [BS, D], F32, isOutput=True)

    with tile.TileContext(nc) as tc:
        with (
            tc.tile_pool(name="const", bufs=1) as const,
            tc.tile_pool(name="xp", bufs=3) as xp,
            tc.tile_pool(name="mid", bufs=2) as mid,
            tc.tile_pool(name="small", bufs=3) as small,
            tc.tile_pool(name="outp", bufs=2) as outp,
            tc.tile_pool(name="pu", bufs=3, space="PSUM") as pup,
            tc.tile_pool(name="psd", bufs=1, space="PSUM") as psp,
            tc.tile_pool(name="po", bufs=2, space="PSUM") as pop,
        ):
            w_sb = const.tile([128, DCH, A], BF16)
            nc.sync.dma_start(w_sb, w_d.rearrange("(c p) a -> p c a", p=128))
            b_sb = const.tile([A, 1], F32)
            nc.sync.dma_start(b_sb, b_d[:, :])
            u_sb = const.tile([A, 1], BF16)
            nc.sync.dma_start(u_sb, u_d[:, :])
            ones_sb = const.tile([128, 1], BF16)
            nc.vector.memset(ones_sb, 1.0)

            if mode == "compute":
                xt_fix = const.tile([128, DCH, T], BF16)
                nc.sync.dma_start(xt_fix, xt_d[0])
                xn_fix = const.tile([128, TJ, D], BF16)
                nc.scalar.dma_start(xn_fix, xn_d[0])
                mk_fix = const.tile([128, TJ], F32)
                nc.scalar.dma_start(mk_fix, mk_d[0])

            for bi in [bi for _ in range(repeat) for bi in range(BS)]:
                # ---- loads ----
                if mode == "compute":
                    xt_sb, xn_sb, mk_sb = xt_fix, xn_fix, mk_fix
                elif mode == "dma1":
                    xt_sb = xp.tile([128, DCH, T], BF16, tag="xt")
                    nc.sync.dma_start(xt_sb, xt_d[bi])
                else:
                    # One DMA per tensor with 4-KiB descriptor runs (measured
                    # sweet spot: ~227 GB/s/core vs ~168 at other sizes).
                    xt_sb = xp.tile([128, DCH, T], BF16, tag="xt")
                    nc.sync.dma_start(_r4k(xt_sb), _r4k(xt_d[bi]))
                    xn_sb = xp.tile([128, TJ, D], BF16, tag="xn")
                    nc.scalar.dma_start(_r4k(xn_sb), _r4k(xn_d[bi]))
                    mk_sb = small.tile([128, TJ], F32, tag="mask")
                    nc.scalar.dma_start(mk_sb, mk_d[bi])
                if mode == "dma":
                    continue
                if mode == "dma1":
                    continue

                # ---- mm1 + tanh: uitT [a=128, T] ----
                uit_sb = mid.tile([A, T], BF16, tag="uit")
                for s in range(TS):
                    pu = pup.tile([128, TSUB], F32, tag="pu")
                    for c in range(DCH):
                        nc.tensor.matmul(
                            pu,
                            w_sb[:, c, :],
                            xt_sb[:, c, ts(s, TSUB)],
                            start=(c == 0),
                            stop=(c == DCH - 1),
                        )
                    nc.scalar.activation(
                        uit_sb[:, ts(s, TSUB)], pu, AFT.Tanh, bias=b_sb
                    )

                # ---- mm2: s[t] with t on partitions: ps [128, TJ] ----
                ps = psp.tile([128, TJ], F32, tag="ps")
                for j in range(TJ):
                    nc.tensor.matmul(
                        ps[:, j : j + 1],
                        uit_sb[:, ts(j, 128)],
                        u_sb,
                        start=True,
                        stop=True,
                    )

                # ---- exp, mask, cast ----
                aitf = small.tile([128, TJ], F32, tag="aitf")
                nc.scalar.activation(aitf, ps, AFT.Exp)
                ait = small.tile([128, TJ], BF16, tag="ait")
                nc.vector.tensor_mul(ait, aitf, mk_sb)

                # ---- denominator ----
                pd = psp.tile([1, TJ], F32, tag="pd")
                nc.tensor.matmul(pd, ones_sb, ait, start=True, stop=True)
                den = small.tile([1, 1], F32, tag="den")
                nc.vector.reduce_sum(den, pd, axis=mybir.AxisListType.X)
                den2 = small.tile([1, 1], F32, tag="den2")
                nc.vector.tensor_scalar_add(den2, den, EPS)
                inv = small.tile([1, 1], F32, tag="inv")
                nc.vector.reciprocal(inv, den2)

                # ---- mm3: out_raw [1, D] ----
                po = pop.tile([1, D], F32, tag="po")
                for j in range(TJ):
                    nc.tensor.matmul(
                        po,
                        ait[:, j : j + 1],
                        xn_sb[:, j, :],
                        start=(j == 0),
                        stop=(j == TJ - 1),
                    )
                out_row = outp.tile([1, D], F32, tag="orow")
                nc.scalar.activation(out_row, po, AFT.Copy, scale=inv)
                nc.sync.dma_start(out_d[bi][None, :], out_row)
    nc.finalize()
    return nc


def _get_nc(repeat=1, mode="full"):
    key = (repeat, mode)
    if key not in _NC_CACHE:
        _NC_CACHE[key] = _build_nc(repeat, mode)
    return _NC_CACHE[key]


def _prepare_in_maps(x, mask, W, b, u):
    x = np.asarray(x, dtype=np.float32)
    mask = np.asarray(mask)
    W = np.asarray(W, dtype=np.float32)
    b = np.asarray(b, dtype=np.float32)
    u = np.asarray(u, dtype=np.float32)

    bf16 = ml_dtypes.bfloat16
    x16 = x.astype(bf16)                                            # [B, T, D]
    # xn[b, p, j, d] = x[b, 128j+p, d]
    xn16 = np.ascontiguousarray(
        x16.reshape(B, TJ, 128, D).transpose(0, 2, 1, 3)
    )                                                               # [B,128,TJ,D]
    # xt[b, p, c, t] = x[b, t, 128c+p]
    xt16 = np.ascontiguousarray(
        x16.transpose(0, 2, 1).reshape(B, DCH, 128, T).transpose(0, 2, 1, 3)
    )                                                               # [B,128,DCH,T]
    # mask -> [B, 128, TJ] with element [b, p, j] = mask[b, 128*j + p]
    mkr = np.ascontiguousarray(
        mask.reshape(B, TJ, 128).transpose(0, 2, 1).astype(np.float32)
    )
    w16 = np.ascontiguousarray(W.astype(bf16))                      # [D, A]
    b32 = np.ascontiguousarray(b.reshape(A, 1).astype(np.float32))  # [A, 1]
    u16 = np.ascontiguousarray(u.reshape(A, 1).astype(bf16))        # [A, 1]

    in_maps = []
    for i in range(NCORES):
        sl = slice(i * BS, (i + 1) * BS)
        in_maps.append(
            {
                "xt": xt16[sl],
                "xn": xn16[sl],
                "maskr": mkr[sl],
                "w": w16,
                "b": b32,
                "u": u16,
            }
        )
    return in_maps


def run(inputs, trace=False, **kwargs):
    """Run the device kernel; returns (output [B, D] f32, BassKernelResults)."""
    nc = _get_nc()
    in_maps = _prepare_in_maps(**inputs)
    res = run_bass_kernel_spmd(
        nc, in_maps, core_ids=list(range(NCORES)), trace=trace, **kwargs
    )
    out = np.concatenate(
        [np.asarray(res.results[i]["out"], dtype=np.float32) for i in range(NCORES)],
        axis=0,
    )
    return out, res


def kernel(x, mask, W, b, u):
    out, _ = run({"x": x, "mask": mask, "W": W, "b": b, "u": u})
    return out


# revision 33
# speedup vs baseline: 1.2431x; 1.2431x over previous
"""Trainium2 Bass kernel for the AttLayer pooling module.

Reference computation (per batch b):
    uit  = tanh(x @ W + bias)            # [T, A]
    ait  = exp(uit @ u) * mask           # [T]
    out  = x^T @ (ait / (sum(ait)+EPS))  # [D]

Distribution: pure data parallel, batch dim B=64 sharded across 8 NeuronCores
(8 batches per core). W/b/u are replicated.

Device dataflow per batch (all matmuls bf16 with f32 PSUM accumulation):
  mm1: uitT[a,t] = sum_d W[d,a] * x[t,d]   lhsT = W d-chunk [128,128] (stationary),
                                           rhs = xT d-chunk [128, t-subtile 512]
  tanh(+bias) on ScalarE, PSUM -> SBUF bf16 (bias b is per-partition since a is
  the partition dim of uitT)
  mm2: s[t] (t on partitions!) via lhsT = uitT t-chunk [a=128, 128], rhs = u [a,1]
  exp on ScalarE -> mask multiply + bf16 cast on VectorE -> ait [128, 16]
  denom: ones^T @ ait -> [1,16] -> reduce -> +EPS -> reciprocal
  mm3: out_raw[1, D] += ait[:, j]^T @ xn[t-chunk j]   (accumulate 16 chunks)
  scale by 1/denom on ScalarE -> out row

The contraction dims differ between mm1 (over D) and mm3 (over T), so the two
matmuls need x in transposed resp. natural layout. Both layouts are staged in
DRAM as bf16, which keeps total HBM read bytes identical to a single f32 copy
of x (the memory roofline is unchanged: 32 MiB/core).
"""

import sys

if "/opt/trn_rl_repo" not in sys.path:
    sys.path.insert(0, "/opt/trn_rl_repo")

import numpy as np
import ml_dtypes

import concourse.bass as bass  # noqa: F401  (registers AP machinery)
import concourse.tile as tile
from concourse import bacc, mybir
from concourse.bass import ts
from concourse.bass_utils import run_bass_kernel_spmd

BF16 = mybir.dt.bfloat16
F32 = mybir.dt.float32
AFT = mybir.ActivationFunctionType

EPS = 1e-7

B, T, D, A = 64, 2048, 512, 128
NCORES = 8
BS = B // NCORES          # 8 batches per core
DCH = D // 128            # 4 d-chunks of 128
TJ = T // 128             # 16 t-chunks of 128
TSUB = 512                # t-subtile width for mm1
TS = T // TSUB            # 4 t-subtiles

_NC_CACHE = {}


def _r4k(ap):
    """Reshape a [128, A, B] AP to 4-KiB (2048 bf16) contiguous runs."""
    return ap.rearrange("p a b -> p (a b)").rearrange("p (k r) -> p k r", r=2048)


def _build_nc(repeat=1, mode="full"):
    """mode: 'full' | 'dma' (loads only) | 'compute' (load once, compute loop)."""
    nc = bacc.Bacc("TRN2", target_bir_lowering=False, debug=False)

    # Pre-swizzled host layouts: a load is one linear DRAM scan, contiguous on
    # both the DRAM and SBUF side (maximal DMA descriptors, sequential DRAM).
    #   xt[b, p, c, t] = x[b, t, 128c+p]   (transposed view, d = 128c+p)
    #   xn[b, p, j, d] = x[b, 128j+p, d]   (natural view,    t = 128j+p)
    xt_d = nc.declare_dram_parameter("xt", [BS, 128, DCH, T], BF16, isOutput=False)
    xn_d = nc.declare_dram_parameter("xn", [BS, 128, TJ, D], BF16, isOutput=False)
    mk_d = nc.declare_dram_parameter("maskr", [BS, 128, TJ], F32, isOutput=False)
    w_d = nc.declare_dram_parameter("w", [D, A], BF16, isOutput=False)
    b_d = nc.declare_dram_parameter("b", [A, 1], F32, isOutput=False)
    u_d = nc.declare_dram_parameter("u", [A, 1], BF16, isOutput=False)
    out_d = nc.declare_dram_parameter("out", [BS, D], F32, isOutput=True)

    with tile.TileContext(nc) as tc:
        with (
            tc.tile_pool(name="const", bufs=1) as const,
            tc.tile_pool(name="xp", bufs=3) as xp,
            tc.tile_pool(name="mid", bufs=2) as mid,
            tc.tile_pool(name="small", bufs=3) as small,
            tc.tile_pool(name="outp", bufs=2) as outp,
            tc.tile_pool(name="pu", bufs=3, space="PSUM") as pup,
            tc.tile_pool(name="psd", bufs=1, space="PSUM") as psp,
            tc.tile_pool(name="po", bufs=2, space="PSUM") as pop,
        ):
            w_sb = const.tile([128, DCH, A], BF16)
            nc.sync.dma_start(w_sb, w_d.rearrange("(c p) a -> p c a", p=128))
            b_sb = const.tile([A, 1], F32)
            nc.sync.dma_start(b_sb, b_d[:, :])
            u_sb = const.tile([A, 1], BF16)
            nc.sync.dma_start(u_sb, u_d[:, :])
            ones_sb = const.tile([128, 1], BF16)
            nc.vector.memset(ones_sb, 1.0)

            if mode == "compute":
                xt_fix = const.tile([128, DCH, T], BF16)
                nc.sync.dma_start(xt_fix, xt_d[0])
                xn_fix = const.tile([128, TJ, D], BF16)
                nc.scalar.dma_start(xn_fix, xn_d[0])
                mk_fix = const.tile([128, TJ], F32)
                nc.scalar.dma_start(mk_fix, mk_d[0])

            for bi in [bi for _ in range(repeat) for bi in range(BS)]:
                # ---- loads ----
                if mode == "compute":
                    xt_sb, xn_sb, mk_sb = xt_fix, xn_fix, mk_fix
                elif mode == "dma1":
                    xt_sb = xp.tile([128, DCH, T], BF16, tag="xt")
                    nc.sync.dma_start(xt_sb, xt_d[bi])
                else:
                    # One DMA per tensor with 4-KiB descriptor runs (measured
                    # sweet spot: ~227 GB/s/core vs ~168 at other sizes),
                    # split across the two HWDGE issue engines. Measured worse
                    # alternatives: finer DMA splits, whole-tile descriptors,
                    # a third stream via gpsimd SWDGE, deeper prefetch bufs.
                    xt_sb = xp.tile([128, DCH, T], BF16, tag="xt")
                    nc.sync.dma_start(_r4k(xt_sb), _r4k(xt_d[bi]))
                    xn_sb = xp.tile([128, TJ, D], BF16, tag="xn")
                    nc.scalar.dma_start(_r4k(xn_sb), _r4k(xn_d[bi]))
                    mk_sb = small.tile([128, TJ], F32, tag="mask")
                    nc.scalar.dma_start(mk_sb, mk_d[bi])
                if mode == "dma":
                    continue
                if mode == "dma1":
                    continue

                # ---- mm1 + tanh: uitT [a=128, T] ----
                uit_sb = mid.tile([A, T], BF16, tag="uit")
                for s in range(TS):
                    pu = pup.tile([128, TSUB], F32, tag="pu")
                    for c in range(DCH):
                        nc.tensor.matmul(
                            pu,
                            w_sb[:, c, :],
                            xt_sb[:, c, ts(s, TSUB)],
                            start=(c == 0),
                            stop=(c == DCH - 1),
                        )
                    nc.scalar.activation(
                        uit_sb[:, ts(s, TSUB)], pu, AFT.Tanh, bias=b_sb
                    )

                # ---- mm2: s[t] with t on partitions: ps [128, TJ] ----
                ps = psp.tile([128, TJ], F32, tag="ps")
                for j in range(TJ):
                    nc.tensor.matmul(
                        ps[:, j : j + 1],
                        uit_sb[:, ts(j, 128)],
                        u_sb,
                        start=True,
                        stop=True,
                    )

                # ---- exp, mask, cast ----
                aitf = small.tile([128, TJ], F32, tag="aitf")
                nc.scalar.activation(aitf, ps, AFT.Exp)
                ait = small.tile([128, TJ], BF16, tag="ait")
                nc.vector.tensor_mul(ait, aitf, mk_sb)

                # ---- denominator ----
                pd = psp.tile([1, TJ], F32, tag="pd")
                nc.tensor.matmul(pd, ones_sb, ait, start=True, stop=True)
                den = small.tile([1, 1], F32, tag="den")
                nc.vector.reduce_sum(den, pd, axis=mybir.AxisListType.X)
                den2 = small.tile([1, 1], F32, tag="den2")
                nc.vector.tensor_scalar_add(den2, den, EPS)
                inv = small.tile([1, 1], F32, tag="inv")
                nc.vector.reciprocal(inv, den2)

                # ---- mm3: out_raw [1, D] ----
                po = pop.tile([1, D], F32, tag="po")
                for j in range(TJ):
                    nc.tensor.matmul(
                        po,
                        ait[:, j : j + 1],
                        xn_sb[:, j, :],
                        start=(j == 0),
                        stop=(j == TJ - 1),
                    )
                out_row = outp.tile([1, D], F32, tag="orow")
                nc.scalar.activation(out_row, po, AFT.Copy, scale=inv)
                nc.sync.dma_start(out_d[bi][None, :], out_row)
    nc.finalize()
    return nc


def _get_nc(repeat=1, mode="full"):
    key = (repeat, mode)
    if key not in _NC_CACHE:
        _NC_CACHE[key] = _build_nc(repeat, mode)
    return _NC_CACHE[key]


def _prepare_in_maps(x, mask, W, b, u):
    x = np.asarray(x, dtype=np.float32)
    mask = np.asarray(mask)
    W = np.asarray(W, dtype=np.float32)
    b = np.asarray(b, dtype=np.float32)
    u = np.asarray(u, dtype=np.float32)

    bf16 = ml_dtypes.bfloat16
    x16 = x.astype(bf16)                                            # [B, T, D]
    # xn[b, p, j, d] = x[b, 128j+p, d]
    xn16 = np.ascontiguousarray(
        x16.reshape(B, TJ, 128, D).transpose(0, 2, 1, 3)
    )                                                               # [B,128,TJ,D]
    # xt[b, p, c, t] = x[b, t, 128c+p]
    xt16 = np.ascontiguousarray(
        x16.transpose(0, 2, 1).reshape(B, DCH, 128, T).transpose(0, 2, 1, 3)
    )                                                               # [B,128,DCH,T]
    # mask -> [B, 128, TJ] with element [b, p, j] = mask[b, 128*j + p]
    mkr = np.ascontiguousarray(
        mask.reshape(B, TJ, 128).transpose(0, 2, 1).astype(np.float32)
    )
    w16 = np.ascontiguousarray(W.astype(bf16))                      # [D, A]
    b32 = np.ascontiguousarray(b.reshape(A, 1).astype(np.float32))  # [A, 1]
    u16 = np.ascontiguousarray(u.reshape(A, 1).astype(bf16))        # [A, 1]

    in_maps = []
    for i in range(NCORES):
        sl = slice(i * BS, (i + 1) * BS)
        in_maps.append(
            {
                "xt": xt16[sl],
                "xn": xn16[sl],
                "maskr": mkr[sl],
                "w": w16,
                "b": b32,
                "u": u16,
            }
        )
    return in_maps


def run(inputs, trace=False, **kwargs):
    """Run the device kernel; returns (output [B, D] f32, BassKernelResults)."""
    nc = _get_nc()
    in_maps = _prepare_in_maps(**inputs)
    res = run_bass_kernel_spmd(
        nc, in_maps, core_ids=list(range(NCORES)), trace=trace, **kwargs
    )
    out = np.concatenate(
        [np.asarray(res.results[i]["out"], dtype=np.float32) for i in range(NCORES)],
        axis=0,
    )
    return out, res


def kernel(x, mask, W, b, u):
    out, _ = run({"x": x, "mask": mask, "W": W, "b": b, "u": u})
    return out
